# revision 10
# baseline (speedup 1.0000x reference)
"""AdaptiveBoxBlur2d on 8 TRN2 NeuronCores (Bass/Tile).

Math: the reference normalizes each (n,c) image, builds a SAT (2D cumsum) and
samples it bilinearly at 4 per-pixel corners (box +/- half-sizes), then
rescales.  Identity: a bilinear-interp difference of a cumsum equals
convolution with a trapezoid window W(u) = clamp01(B'-u+1) - clamp01(A'-u+1),
A' = clamp(c0 - (k+1)*s, 0, D-1), B' = clamp(c0 + (k-1)*s, 0, D-1),
s = (D-1)/(2D).  The 2D op is the per-pixel product window
sum_{u,v} Wy(u)Wx(v) xn[u,v], support |u-h|<=8, |v-w|<=8 (k in [1,16)).
This gives an exact 17x17 per-pixel-weighted window sum with ANALYTIC
weights -- no data-dependent gathers, which Trainium has no fast path for.

Sharding: data-parallel over the 16 (n,c) channel-images, 2 per core
(cores 0-3 -> image 0, cores 4-7 -> image 1).  No collectives.
"""

import sys
from contextlib import ExitStack

import numpy as np

sys.path.insert(0, "/opt/trn_rl_repo")

N, C, H, W = 2, 8, 1024, 1024
EPS = 1e-5
SC = (W - 1) / (2.0 * W)  # 0.49951171875 (same for H)
BAND = 112                # output rows per band; 128-row working tile, 8+8 halo
NBANDS = (H + BAND - 1) // BAND  # 10
PADW = 8
WP = W + 2 * PADW
NCH = 2                   # channels per core

_COMPILED = None


def build_bass():
    import concourse.bass as bass
    import concourse.tile as tile
    from concourse import bacc, mybir

    fp32 = mybir.dt.float32
    bf16 = mybir.dt.bfloat16
    AX = mybir.AxisListType
    OP = mybir.AluOpType
    AF = mybir.ActivationFunctionType

    nc = bacc.Bacc("TRN2", target_bir_lowering=False, debug=False)

    x_d = nc.dram_tensor("x", [NCH, H, W], fp32, kind="ExternalInput")
    ks_d = nc.dram_tensor("kernel_sizes", [H, W, 2], fp32, kind="ExternalInput")
    out_d = nc.dram_tensor("out", [NCH, H, W], fp32, kind="ExternalOutput")

    with tile.TileContext(nc) as tc, ExitStack() as ctx:
        singles = ctx.enter_context(tc.tile_pool(name="singles", bufs=1))
        coords_p = ctx.enter_context(tc.tile_pool(name="coords", bufs=1))
        wx_p = ctx.enter_context(tc.tile_pool(name="wx", bufs=1))
        work_p = ctx.enter_context(tc.tile_pool(name="work", bufs=2))
        sh_p = ctx.enter_context(tc.tile_pool(name="sh", bufs=2))
        tmp_p = ctx.enter_context(tc.tile_pool(name="tmp", bufs=2))
        acc_p = ctx.enter_context(tc.tile_pool(name="acc", bufs=2))

        # ---- constants ----
        iota_i = singles.tile([128, 1], mybir.dt.int32)
        nc.gpsimd.iota(iota_i, pattern=[[0, 1]], base=0, channel_multiplier=1)
        iota_col = singles.tile([128, 1], fp32)
        nc.vector.tensor_copy(out=iota_col, in_=iota_i)
        wrow_i = singles.tile([128, W], mybir.dt.int32)
        nc.gpsimd.iota(wrow_i, pattern=[[1, W]], base=0, channel_multiplier=0)
        wrow = singles.tile([128, W], fp32)
        nc.vector.tensor_copy(out=wrow, in_=wrow_i)
        ones_col = singles.tile([128, 1], fp32)
        nc.vector.memset(ones_col, 1.0)
        # per-channel scalars on partition 0: [s1, nb, s2, m] x NCH
        scal = singles.tile([1, NCH * 4], fp32)
        bcast = singles.tile([128, NCH * 4], fp32)

        # ---- pass 1: per-channel mean/std over the full image ----
        p1 = ExitStack()
        stats_p = p1.enter_context(tc.tile_pool(name="stats", bufs=2))
        psum_p = p1.enter_context(tc.tile_pool(name="ps", bufs=2, space="PSUM"))
        xload_p = p1.enter_context(tc.tile_pool(name="xload", bufs=3))
        for ch in range(NCH):
            s_acc = stats_p.tile([128, 2], fp32)
            nc.vector.memset(s_acc, 0.0)
            for t in range(H // 128):
                xt = xload_p.tile([128, W], fp32)
                nc.sync.dma_start(out=xt, in_=x_d[ch, t * 128:(t + 1) * 128, :])
                red = stats_p.tile([128, 2], fp32)
                nc.vector.tensor_reduce(red[:, 0:1], xt, axis=AX.X, op=OP.add)
                sq = xload_p.tile([128, W], fp32)
                nc.scalar.square(sq, xt)
                nc.vector.tensor_reduce(red[:, 1:2], sq, axis=AX.X, op=OP.add)
                nc.vector.tensor_tensor(s_acc, s_acc, red, OP.add)
            ps = psum_p.tile([1, 2], fp32)
            nc.tensor.matmul(out=ps, lhsT=ones_col, rhs=s_acc, start=True, stop=True)
            tot = stats_p.tile([1, 2], fp32)
            nc.vector.tensor_copy(out=tot, in_=ps)
            nel = float(H * W)
            m = stats_p.tile([1, 1], fp32)
            nc.scalar.mul(m, tot[:, 0:1], 1.0 / nel)
            t1 = stats_p.tile([1, 1], fp32)
            nc.vector.tensor_tensor(t1, tot[:, 0:1], m, OP.mult)
            t2 = stats_p.tile([1, 1], fp32)
            nc.vector.tensor_tensor(t2, tot[:, 1:2], t1, OP.subtract)
            var = stats_p.tile([1, 1], fp32)
            nc.scalar.mul(var, t2, 1.0 / (nel - 1.0))
            std = stats_p.tile([1, 1], fp32)
            nc.scalar.sqrt(std, var)
            sp = stats_p.tile([1, 1], fp32)
            nc.vector.tensor_scalar(out=sp, in0=std, scalar1=EPS, scalar2=None, op0=OP.add)
            s1 = stats_p.tile([1, 1], fp32)
            nc.vector.reciprocal(out=s1, in_=sp)
            nb = stats_p.tile([1, 1], fp32)
            nc.vector.tensor_tensor(nb, m, s1, OP.mult)
            nc.vector.tensor_copy(out=scal[:, ch * 4 + 0:ch * 4 + 1], in_=s1)
            nc.vector.tensor_scalar(out=scal[:, ch * 4 + 1:ch * 4 + 2], in0=nb,
                                    scalar1=-1.0, scalar2=None, op0=OP.mult)
            nc.vector.tensor_copy(out=scal[:, ch * 4 + 2:ch * 4 + 3], in_=std)
            nc.vector.tensor_copy(out=scal[:, ch * 4 + 3:ch * 4 + 4], in_=m)

        nc.gpsimd.partition_broadcast(bcast, scal)
        p1.close()

        def clamp01_shift(dst, src, shift):
            # dst = clamp01(src + shift)
            nc.vector.tensor_scalar(out=dst, in0=src, scalar1=float(shift),
                                    scalar2=0.0, op0=OP.add, op1=OP.max)
            nc.vector.tensor_scalar(out=dst, in0=dst, scalar1=1.0, scalar2=None,
                                    op0=OP.min)

        # ---- pass 2: banded trapezoid convolution ----
        for b in range(NBANDS):
            r0 = b * BAND
            nrows = min(BAND, H - r0)
            w0 = r0 - 8
            v0 = max(0, -w0)
            v1 = min(128, H - w0)

            # kernel_sizes for output rows -> partitions 8..8+nrows
            kst = coords_p.tile([128, W, 2], fp32)
            nc.sync.dma_start(out=kst[8:8 + nrows], in_=ks_d[r0:r0 + nrows, :, :])
            ksx = coords_p.tile([128, W], fp32)
            ksy = coords_p.tile([128, W], fp32)
            nc.vector.tensor_copy(out=ksx, in_=kst[:, :, 0])
            nc.vector.tensor_copy(out=ksy, in_=kst[:, :, 1])

            hcol = coords_p.tile([128, 1], fp32)
            nc.vector.tensor_scalar(out=hcol, in0=iota_col, scalar1=float(w0),
                                    scalar2=None, op0=OP.add)

            # window ends relative to the pixel (x axis: pos = wrow tensor)
            bxr = coords_p.tile([128, W], fp32)
            axr = coords_p.tile([128, W], fp32)
            tx = tmp_p.tile([128, W], fp32, bufs=1)
            nc.scalar.mul(tx, ksx, SC)                               # ksx*SC
            nc.vector.tensor_tensor(bxr, tx, wrow, OP.add)
            nc.vector.tensor_scalar(out=bxr, in0=bxr, scalar1=-SC, scalar2=None, op0=OP.add)
            nc.vector.tensor_scalar(out=bxr, in0=bxr, scalar1=0.0,
                                    scalar2=float(W - 1), op0=OP.max, op1=OP.min)
            nc.vector.tensor_tensor(bxr, bxr, wrow, OP.subtract)
            nc.scalar.mul(tx, ksx, -SC)
            nc.vector.tensor_tensor(axr, tx, wrow, OP.add)
            nc.vector.tensor_scalar(out=axr, in0=axr, scalar1=-SC, scalar2=None, op0=OP.add)
            nc.vector.tensor_scalar(out=axr, in0=axr, scalar1=0.0,
                                    scalar2=float(W - 1), op0=OP.max, op1=OP.min)
            nc.vector.tensor_tensor(axr, axr, wrow, OP.subtract)
            # y axis: pos = hcol per-partition scalar
            byr = coords_p.tile([128, W], fp32)
            ayr = coords_p.tile([128, W], fp32)
            nc.scalar.mul(tx, ksy, SC)
            nc.vector.tensor_scalar(out=byr, in0=tx, scalar1=hcol, scalar2=-SC,
                                    op0=OP.add, op1=OP.add)
            nc.vector.tensor_scalar(out=byr, in0=byr, scalar1=0.0,
                                    scalar2=float(H - 1), op0=OP.max, op1=OP.min)
            nc.vector.tensor_scalar(out=byr, in0=byr, scalar1=hcol, scalar2=None,
                                    op0=OP.subtract)
            nc.scalar.mul(tx, ksy, -SC)
            nc.vector.tensor_scalar(out=ayr, in0=tx, scalar1=hcol, scalar2=-SC,
                                    op0=OP.add, op1=OP.add)
            nc.vector.tensor_scalar(out=ayr, in0=ayr, scalar1=0.0,
                                    scalar2=float(H - 1), op0=OP.max, op1=OP.min)
            nc.vector.tensor_scalar(out=ayr, in0=ayr, scalar1=hcol, scalar2=None,
                                    op0=OP.subtract)

            # 1/(area+eps)
            rar = coords_p.tile([128, W], fp32)
            nc.vector.tensor_tensor(rar, ksx, ksy, OP.mult)
            nc.vector.tensor_scalar(out=rar, in0=rar, scalar1=EPS, scalar2=None, op0=OP.add)
            nc.vector.reciprocal(out=rar, in_=rar)

            # Wx planes (17), duplicated for both channels, bf16 storage
            wx = wx_p.tile([128, 17, W], bf16)
            qb = tmp_p.tile([128, W], fp32, bufs=1)
            qa = tmp_p.tile([128, W], fp32, bufs=1)
            wsrc = tmp_p.tile([128, W], fp32, bufs=1)
            for di, dv in enumerate(range(-8, 9)):
                clamp01_shift(qb, bxr, 1 - dv)
                clamp01_shift(qa, axr, 1 - dv)
                nc.vector.tensor_tensor(wsrc, qb, qa, OP.subtract)
                nc.vector.tensor_copy(out=wx[:, di, :], in_=wsrc)

            # normalized working tile, both channels: [128, NCH, WP]
            workf = work_p.tile([128, NCH, WP], fp32)
            nc.vector.memset(workf, 0.0)
            work = work_p.tile([128, NCH, WP], bf16)
            for ch in range(NCH):
                nc.sync.dma_start(out=workf[v0:v1, ch, PADW:PADW + W],
                                  in_=x_d[ch, w0 + v0:w0 + v1, :])
                nc.scalar.activation(out=work[:, ch, :], in_=workf[:, ch, :],
                                     func=AF.Identity,
                                     bias=bcast[:, ch * 4 + 1:ch * 4 + 2],
                                     scale=bcast[:, ch * 4 + 0:ch * 4 + 1])

            acc = acc_p.tile([128, NCH, W], fp32)
            nc.vector.memset(acc, 0.0)

            for r in range(-8, 9):
                a = max(0, -r)
                bb = 128 - max(0, r)
                if r == 0:
                    sha = work
                else:
                    sha = sh_p.tile([128, NCH, WP], bf16)
                    nc.sync.dma_start(out=sha[a:bb], in_=work[a + r:bb + r])
                shb = sh_p.tile([128, NCH, WP], bf16, name="shb")
                nc.sync.dma_start(out=shb[a:bb, :, 0:WP - 1],
                                  in_=work[a + r:bb + r, :, 1:WP])
                wy = tmp_p.tile([128, W], fp32)
                clamp01_shift(qb, byr, 1 - r)
                clamp01_shift(qa, ayr, 1 - r)
                nc.vector.tensor_tensor(wy, qb, qa, OP.subtract)

                for chi in range(NCH):
                    tmpm = acc_p.tile([128, W], bf16)
                    t2m = acc_p.tile([128, W], bf16)
                    t3m = acc_p.tile([128, W], fp32)
                    first = True
                    for di, dv in enumerate(range(-8, 9)):
                        if (PADW + dv) % 2 == 0:
                            srcv = sha[:, chi, PADW + dv:PADW + dv + W]
                        else:
                            srcv = shb[:, chi, PADW + dv - 1:PADW + dv - 1 + W]
                        if first:
                            nc.vector.tensor_tensor(tmpm, wx[:, di], srcv, OP.mult)
                            first = False
                        else:
                            nc.vector.tensor_tensor(t2m, wx[:, di], srcv, OP.mult)
                            nc.vector.tensor_tensor(tmpm, tmpm, t2m, OP.add)
                    nc.vector.tensor_tensor(t3m, wy, tmpm, OP.mult)
                    nc.vector.tensor_tensor(acc[:, chi], acc[:, chi], t3m, OP.add)

            for ch in range(NCH):
                nc.vector.tensor_tensor(acc[:, ch], acc[:, ch], rar, OP.mult)
                nc.scalar.activation(out=acc[:, ch], in_=acc[:, ch], func=AF.Identity,
                                     bias=bcast[:, ch * 4 + 3:ch * 4 + 4],
                                     scale=bcast[:, ch * 4 + 2:ch * 4 + 3])
                nc.sync.dma_start(out=out_d[ch, r0:r0 + nrows, :],
                                  in_=acc[8:8 + nrows, ch])

    nc.compile()
    return nc


LAST_EXEC_NS = None
LAST_PROFILE = None


def kernel(x: np.ndarray, kernel_sizes: np.ndarray, _trace: bool = False) -> np.ndarray:
    global _COMPILED, LAST_EXEC_NS, LAST_PROFILE
    from concourse import bass_utils

    if _COMPILED is None:
        _COMPILED = build_bass()
    nc = _COMPILED

    x = np.ascontiguousarray(x, dtype=np.float32)
    ks = np.ascontiguousarray(kernel_sizes, dtype=np.float32)
    in_maps = []
    for core in range(8):
        n = core // 4
        c0 = (core % 4) * NCH
        in_maps.append({
            "x": np.ascontiguousarray(x[n, c0:c0 + NCH]),
            "kernel_sizes": ks[n],
        })
    res = bass_utils.run_bass_kernel_spmd(nc, in_maps, core_ids=list(range(8)),
                                          trace=_trace)
    LAST_EXEC_NS = res.exec_time_ns
    LAST_PROFILE = res.profile_json
    out = np.empty((N, C, H, W), dtype=np.float32)
    for core in range(8):
        n = core // 4
        c0 = (core % 4) * NCH
        out[n, c0:c0 + NCH] = res.results[core]["out"].reshape(NCH, H, W)
    return out
